# revision 1
# baseline (speedup 1.0000x reference)
"""Fused Trainium2 kernel for the ConvPolicy8 tiny CNN (batch=1).

The whole ~12-op conv/deconv chain runs as ONE Bass/Tile kernel on a
single NeuronCore.  The host packs *everything* the device needs into a
single [14, 190] f32 tensor (one DMA, since each DMA costs ~2us
end-to-end on this part):
  * cols 0:168   every conv/deconv k-slice pre-transposed into the lhsT
                 layout the PE wants, plus biases and the d4 bias row.
  * cols 168:180 the zero-padded jcat block and the jlrs block for the
                 final concat -- read directly as matmul rhs operands.
  * cols 180:183 the quaternion reordered for the atan2 lanes + obs.
  * col  183     zeros (bias operand for the atan2 activations).
  * cols 184:188 ones (rhs row that folds d4's bias into its matmul).
On device each layer is a group of accumulating matmuls (PE) + one
fused bias+tanh activation (ScalarE).  Channel concats (e1's
[conv|psi|obsd] input, d4's [upsample|jlrs|bias] input) are extra
accumulating matmuls.  atan2 uses Arctan/Sign activations (same ACT
table set as Tanh -> exactly one table load, warmed by a dummy
activation at t=0) and psi = at_a + at_b is folded into the e1b weight
slice by duplicating the psi row.  d4 accumulates its bias in PSUM, so
the result DMAs straight from PSUM to DRAM; the final reshape(24)[2:]
happens on host.
"""

import numpy as np

import concourse.bass as bass
import concourse.mybir as mybir
import concourse.tile as tile
from concourse import bacc
from concourse.bass_utils import run_bass_kernel_spmd

AF = mybir.ActivationFunctionType
ALU = mybir.AluOpType
F32 = mybir.dt.float32

# matmul parts: name -> (Cin, Cout, K).  All in effective-convolution
# form (deconvs become convs with flipped/transposed kernels).
_PARTS = {
    "c1": (12, 4, 3),
    "c2": (4, 8, 3),
    "c3": (8, 8, 3),
    "c4": (8, 8, 2),
    "e1a": (8, 8, 1),
    "e1b": (2, 8, 1),
    "e1c": (1, 8, 1),
    "e2": (8, 8, 1),
    "d1": (8, 4, 3),
    "d2": (4, 4, 3),
    "d3": (4, 8, 3),
    "d4a": (8, 6, 3),
    "d4b": (6, 6, 3),
    "d4c": (1, 6, 1),  # bias row: lhsT = b_d4, rhs = ones
}
# bias columns for the tanh layers
_BIAS = {
    "c1": 4, "c2": 8, "c3": 8, "c4": 8, "e1": 8,
    "e2": 8, "d1": 4, "d2": 4, "d3": 8,
}

_WROWS = 14


def _wlayout():
    woffs, boffs, col = {}, {}, 0
    for name, (_, cout, k) in _PARTS.items():
        woffs[name] = col
        col += k * cout
    for name in _BIAS:
        boffs[name] = col
        col += 1
    lay = {"jcat": col, "catlo": col + 6, "quat": col + 12, "zbias": col + 15,
           "ones": col + 16}
    return woffs, boffs, lay, col + 20


_WOFFS, _BOFFS, _LAY, _WCOLS = _wlayout()


def pack_all(inp):
    """The single packed input [14, _WCOLS]."""
    W = np.zeros((_WROWS, _WCOLS), np.float32)

    def put(name, j, mat):
        cout = _PARTS[name][1]
        col = _WOFFS[name] + j * cout
        W[: mat.shape[0], col : col + mat.shape[1]] = mat

    # Conv1d weights are [Cout, Cin, K]; lhsT_k = w[:, :, k].T
    for name in ("c1", "c2", "c3", "c4", "e2"):
        w = np.asarray(inp["w_" + name])
        for j in range(_PARTS[name][2]):
            put(name, j, w[:, :, j].T)

    # e1: [8, 10, 1] with in-ch 8 = psi, 9 = obsd.  Split into the conv
    # part, the two duplicated psi-lane rows, and the obsd row.
    w = np.asarray(inp["w_e1"])
    put("e1a", 0, w[:, 0:8, 0].T)
    put("e1b", 0, np.stack([w[:, 8, 0], w[:, 8, 0]]))
    put("e1c", 0, w[:, 9, 0][None, :])

    # ConvTranspose1d weights are [Cin, Cout, K]; lhsT_k = w[:, :, K-1-k]
    for name in ("d1", "d2", "d3"):
        w = np.asarray(inp["w_" + name])
        k = _PARTS[name][2]
        for j in range(k):
            put(name, j, w[:, :, k - 1 - j])
    w = np.asarray(inp["w_d4"])
    for j in range(3):
        put("d4a", j, w[0:8, :, 2 - j])
        put("d4b", j, w[8:14, :, 2 - j])
    put("d4c", 0, np.asarray(inp["b_d4"])[None, :])

    for name, cout in _BIAS.items():
        W[:cout, _BOFFS[name]] = np.asarray(inp["b_" + name])

    # x-derived blocks
    x = np.asarray(inp["x"], np.float32)[0]
    z2 = np.zeros(2, np.float32)
    jl = np.concatenate([z2, x[7:29]]).reshape(6, 4)
    jd = np.concatenate([z2, x[35:57]]).reshape(6, 4)
    o = _LAY["jcat"]
    W[0:6, o + 1 : o + 5] = jl
    W[6:12, o + 1 : o + 5] = jd
    o = _LAY["catlo"]
    W[0:6, o + 1 : o + 5] = jl
    # atan2 lanes: row p0 = [n=qz, d=qw, obsd], row p1 = [n=qx, d=qy, 0]
    o = _LAY["quat"]
    W[0, o : o + 3] = [x[6], x[3], x[34]]
    W[1, o : o + 2] = [x[4], x[5]]
    W[0, _LAY["ones"] : _LAY["ones"] + 4] = 1.0
    return W


def build():
    """Build + compile the Bass module. Returns the Bacc instance."""
    nc = bacc.Bacc("TRN2", target_bir_lowering=False, debug=False)
    wpack_t = nc.dram_tensor("wpack", [_WROWS, _WCOLS], F32, kind="ExternalInput")
    out_t = nc.dram_tensor("out", [6, 4], F32, kind="ExternalOutput")

    with tile.TileContext(nc) as tc:
        with (
            tc.tile_pool(name="sb", bufs=1) as sb,
            tc.tile_pool(name="pp", bufs=3, space="PSUM") as pp,
            tc.tile_pool(name="pp4", bufs=1, space="PSUM") as pp4,
        ):
            w = sb.tile([_WROWS, _WCOLS], F32)
            fm1 = sb.tile([4, 6], F32)  # c2 input, pad 1
            fm2 = sb.tile([8, 4], F32)  # c3 input
            fm3 = sb.tile([8, 2], F32)  # c4 input
            emb = sb.tile([8, 1], F32)  # e1a input (conv channels)
            pex = sb.tile([2, 1], F32)  # e1b input (the two atan2 lanes)
            emb1 = sb.tile([8, 1], F32)  # e2 input
            emb2 = sb.tile([8, 5], F32)  # d1 input, pad 2
            dc1 = sb.tile([4, 5], F32)  # d2 input, pad 1
            dc2 = sb.tile([4, 5], F32)  # d3 input, pad 1
            cat_hi = sb.tile([8, 6], F32)  # d4a input, pad 1 (upsampled d3)
            pss = sb.tile([2, 8], F32)  # atan2 scratch
            wrm = sb.tile([1, 2], F32)  # ACT table warm-up
            stg = sb.tile([6, 4], F32)  # d4 output staging (DMA can't read PSUM)

            # The single input DMA (HWDGE).
            nc.sync.dma_start(w[:, :], wpack_t[:, :])

            # Warm the ACT table set immediately: the table load overlaps
            # the input DMA instead of stalling the first layer.  Arctan
            # pins the set choice to sigmoid_and_others (which also holds
            # Tanh and Sign) -> exactly one table load in the kernel.  The
            # framework's const-0.0 region is already zeroed in the
            # preamble, so no extra memset or DMA dependency.
            zero_c = nc.const_aps.aps[(F32, 0.0)]
            nc.scalar.activation(
                wrm[:, 1:2], zero_c[0:1, 0:1], AF.Arctan, bias=zero_c[0:1, 0:1]
            )

            # Zero the padded borders of intermediate tiles (GPSIMD, off
            # the critical engines).
            nc.gpsimd.memset(fm1[:, :], 0.0)
            nc.gpsimd.memset(emb2[:, :], 0.0)
            nc.gpsimd.memset(dc1[:, :], 0.0)
            nc.gpsimd.memset(dc2[:, :], 0.0)
            nc.gpsimd.memset(cat_hi[:, :], 0.0)

            # psi = atan2(qz,qw) + atan2(qx,qy), two lanes on partitions 0:2.
            # atan2(n,d) = arctan(n/d) + pi*sign(n)*[d<0]
            q = _LAY["quat"]
            n_ap, d_ap = w[0:2, q : q + 1], w[0:2, q + 1 : q + 2]
            nc.vector.reciprocal(pss[0:2, 0:1], d_ap)
            nc.vector.tensor_tensor(pss[0:2, 1:2], n_ap, pss[0:2, 0:1], ALU.mult)
            nc.scalar.activation(
                pss[0:2, 2:3], pss[0:2, 1:2], AF.Arctan, bias=zero_c[0:2, 0:1]
            )
            nc.scalar.activation(
                pss[0:2, 3:4], n_ap, AF.Sign, bias=zero_c[0:2, 0:1]
            )
            nc.vector.tensor_scalar(pss[0:2, 4:5], d_ap, 0.0, None, ALU.is_lt)
            nc.vector.tensor_scalar(
                pss[0:2, 5:6],
                pss[0:2, 4:5],
                pss[0:2, 3:4],
                float(np.pi),
                ALU.mult,
                ALU.mult,
            )
            nc.vector.tensor_tensor(
                pex[0:2, 0:1], pss[0:2, 2:3], pss[0:2, 5:6], ALU.add
            )

            def mm(ps, pname, in_tile, off, j, lout, start=False, stop=False):
                cin, cout, _ = _PARTS[pname]
                wof = _WOFFS[pname]
                nc.tensor.matmul(
                    ps[0:cout, 0:lout],
                    w[0:cin, wof + j * cout : wof + (j + 1) * cout],
                    in_tile[0:cin, off + j : off + j + lout],
                    start=start,
                    stop=stop,
                )

            def layer(parts, lout, out_ap=None, bias_name=None):
                """parts: list of (part_name, tile, col_off); each element
                contributes K accumulating matmuls into a shared PSUM tile.
                With bias_name, applies bias+tanh into out_ap; otherwise
                returns the PSUM tile."""
                cout = _PARTS[parts[0][0]][1]
                ps = pp.tile([cout, lout], F32, tag="ps")
                nmm = sum(_PARTS[p][2] for p, _, _ in parts)
                i = 0
                for pname, in_tile, off in parts:
                    k = _PARTS[pname][2]
                    for j in range(k):
                        mm(ps, pname, in_tile, off, j, lout, i == 0, i == nmm - 1)
                        i += 1
                if bias_name is not None:
                    bias = w[0:cout, _BOFFS[bias_name] : _BOFFS[bias_name] + 1]
                    nc.scalar.activation(out_ap, ps[0:cout, 0:lout], AF.Tanh, bias=bias)
                return ps

            layer([("c1", w, _LAY["jcat"])], 4, fm1[0:4, 1:5], "c1")
            # d4's jlrs/bias matmuls depend only on the input DMA -- run
            # them now, while the PE would otherwise idle, so only the
            # three d4a matmuls remain on the critical tail.
            ps4 = pp4.tile([6, 4], F32, tag="d4")
            mm(ps4, "d4b", w, _LAY["catlo"], 0, 4, start=True)
            mm(ps4, "d4b", w, _LAY["catlo"], 1, 4)
            mm(ps4, "d4b", w, _LAY["catlo"], 2, 4)
            mm(ps4, "d4c", w, _LAY["ones"], 0, 4)
            layer([("c2", fm1, 0)], 4, fm2[0:8, 0:4], "c2")
            layer([("c3", fm2, 0)], 2, fm3[0:8, 0:2], "c3")
            layer([("c4", fm3, 0)], 1, emb[0:8, 0:1], "c4")
            layer(
                [("e1a", emb, 0), ("e1b", pex, 0), ("e1c", w, _LAY["quat"] + 2)],
                1, emb1[0:8, 0:1], "e1",
            )
            layer([("e2", emb1, 0)], 1, emb2[0:8, 2:3], "e2")
            layer([("d1", emb2, 0)], 3, dc1[0:4, 1:4], "d1")
            layer([("d2", dc1, 0)], 3, dc2[0:4, 1:4], "d2")
            layer([("d3", dc2, 0)], 3, cat_hi[0:8, 2:5], "d3")
            # nearest-neighbor upsample [0,0,1,2] duplicates d3's first
            # column (cat cols 1 and 2 are equal).  Instead of a second
            # activation writing col 1, leave it zero and add the
            # duplicate's contribution with two correction matmuls:
            # out[:,0] += W_{k=1} . u0 and out[:,1] += W_{k=0} . u0, whose
            # weight slices already sit in the pack as d4a's j=1 / j=0.
            mm(ps4, "d4a", cat_hi, 0, 2, 4)
            mm(ps4, "d4a", cat_hi, 0, 0, 4)
            mm(ps4, "d4a", cat_hi, 0, 1, 4)
            wo = _WOFFS["d4a"]
            nc.tensor.matmul(
                ps4[0:6, 0:1], w[0:8, wo + 6 : wo + 12], cat_hi[0:8, 2:3],
                start=False, stop=False,
            )
            nc.tensor.matmul(
                ps4[0:6, 1:2], w[0:8, wo : wo + 6], cat_hi[0:8, 2:3],
                start=False, stop=True,
            )

            # d4 result (bias already accumulated in PSUM) -> SBUF -> DRAM.
            nc.vector.tensor_copy(stg[0:6, 0:4], ps4[0:6, 0:4])
            nc.sync.dma_start(out_t[:, :], stg[0:6, 0:4])

    nc.compile()
    return nc


_NC = None


def _get_nc():
    global _NC
    if _NC is None:
        _NC = build()
    return _NC


def make_in_map(inputs):
    return {"wpack": pack_all(inputs)}


def kernel(**inputs) -> np.ndarray:
    nc = _get_nc()
    res = run_bass_kernel_spmd(nc, [make_in_map(inputs)], core_ids=[0])
    acts = np.asarray(res.results[0]["out"], np.float32).reshape(1, 24)
    return np.ascontiguousarray(acts[:, 2:])



# revision 4
# speedup vs baseline: 1.3268x; 1.3268x over previous
"""Fused Trainium2 kernel for the ConvPolicy8 tiny CNN (batch=1).

The whole ~12-op conv/deconv chain runs as ONE Bass/Tile kernel on a
single NeuronCore, in a "tall vector" formulation: every layer's
activation tensor [C, L] is kept as a single SBUF column [C*L, 1]
(partition dim = (position, channel)), and every conv/deconv layer is
ONE width-1 matmul out[CL',1] = lhsT.T @ in[CL,1] where lhsT is the
layer's block-banded weight matrix, pre-built on the host from the
conv kernels (padding, kernel flips and the nearest-neighbor upsample
duplication are all folded into the matrix).  Each tanh is then ONE
activation instruction whose operands are all [N,1] per-partition
scalars.

Host-side pack: a single [48, C] f32 tensor (one DMA) containing all
the lhsT blocks, bias columns, the x-derived tall vectors (jcat, jlrs)
and the quaternion lanes for the on-device atan2.
"""

import numpy as np

import concourse.bass as bass
import concourse.mybir as mybir
import concourse.tile as tile
from concourse import bacc
from concourse.bass_utils import run_bass_kernel_spmd

AF = mybir.ActivationFunctionType
ALU = mybir.AluOpType
F32 = mybir.dt.float32

_ROWS = 48

# ---------------------------------------------------------------------------
# layout: skyline-pack named (h, w) blocks into a [48, C] tensor
# ---------------------------------------------------------------------------

# tall sizes per layer: (in_rows, out_cols)
_MM = {
    "c1": (48, 16),
    "c2": (16, 32),
    "c3": (32, 16),
    "c4": (16, 8),
    "e1a": (8, 8),
    "e1b": (2, 8),
    "e1c": (1, 8),
    "e2": (8, 8),
    "d1": (8, 12),
    "d2": (12, 12),
    "d3": (12, 24),
    "d4a": (24, 24),
    "d4b": (24, 24),
    "d4c": (1, 24),
}
# bias column heights (tall CL of the layer output)
_BIAS = {"c1": 16, "c2": 32, "c3": 16, "c4": 8, "e1": 8, "e2": 8,
         "d1": 12, "d2": 12, "d3": 24}


def _layout():
    blocks = []
    for n, (h, w) in _MM.items():
        blocks.append((n, h, w))
    for n, h in _BIAS.items():
        blocks.append(("b_" + n, h, 1))
    blocks.append(("jt", 48, 1))     # jcat tall [48,1]
    blocks.append(("jlt", 24, 1))    # jlrs tall [24,1]
    blocks.append(("quat", 2, 2))    # [[qz, qw], [qx, qy]]
    blocks.append(("obsd", 1, 1))    # x[34]
    blocks.append(("ones", 1, 1))    # 1.0 (d4 bias rhs)
    # matmul operands need base partition 0 (PE constraint: 0/32/64 only),
    # so every block sits at row 0 in its own column range.
    pos = {}
    col = 0
    for n, _h, w in blocks:
        pos[n] = (0, col)
        col += w
    return pos, col


_POS, _COLS = _layout()


# ---------------------------------------------------------------------------
# host pack
# ---------------------------------------------------------------------------


def _conv_lhsT(w, cin, lin, cout, lout, shift):
    """Block-banded lhsT [cin*lin, cout*lout] for an effective conv:
    out[o,p] = sum_{i,k} w[o,i,k] * in[i, p+k+shift] (tall idx = pos*C+ch)."""
    k = w.shape[2]
    m = np.zeros((cin * lin, cout * lout), np.float32)
    for p in range(lout):
        for kk in range(k):
            pp = p + kk + shift
            if 0 <= pp < lin:
                m[pp * cin : (pp + 1) * cin, p * cout : (p + 1) * cout] += w[:, :, kk].T
    return m


def pack_all(inp):
    W = np.zeros((_ROWS, _COLS), np.float32)

    def put(name, mat):
        r, c = _POS[name]
        W[r : r + mat.shape[0], c : c + mat.shape[1]] = mat

    g = lambda n: np.asarray(inp[n], np.float32)

    # conv layers (PyTorch Conv1d [O,I,K]): pad=1 -> shift -1; pad=0 -> 0
    put("c1", _conv_lhsT(g("w_c1"), 12, 4, 4, 4, -1))
    put("c2", _conv_lhsT(g("w_c2"), 4, 4, 8, 4, -1))
    put("c3", _conv_lhsT(g("w_c3"), 8, 4, 8, 2, 0))
    put("c4", _conv_lhsT(g("w_c4"), 8, 2, 8, 1, 0))
    # deconvs (ConvTranspose1d [I,O,K], stride 1): effective conv with
    # flipped/transposed kernel, eff pad = K-1-pad -> shift = -(K-1-pad)
    dw = lambda n: np.flip(g(n), -1).transpose(1, 0, 2)  # -> [O,I,K]
    put("d1", _conv_lhsT(dw("w_d1"), 8, 1, 4, 3, -2))
    put("d2", _conv_lhsT(dw("w_d2"), 4, 3, 4, 3, -1))
    put("d3", _conv_lhsT(dw("w_d3"), 4, 3, 8, 3, -1))

    # e1: [8,10,1]; in-ch 8 = psi (duplicated-lane trick), 9 = obsd
    we1 = g("w_e1")
    put("e1a", we1[:, 0:8, 0].T)
    put("e1b", np.stack([we1[:, 8, 0], we1[:, 8, 0]]))
    put("e1c", we1[:, 9, 0][None, :])
    put("e2", g("w_e2")[:, :, 0].T)

    # d4: eff conv (14 -> 6, K=3, eff pad 1) on [up(d3) ; jlrs], OUT tall
    # index o*4+p (channel-major, to match the reference flatten).
    wd4 = dw("w_d4")  # [6, 14, 3]
    up_idx = [0, 0, 1, 2]
    m4a = np.zeros((24, 24), np.float32)
    m4b = np.zeros((24, 24), np.float32)
    for p in range(4):
        for kk in range(3):
            pp = p + kk - 1
            if not (0 <= pp < 4):
                continue
            for o in range(6):
                # up(d3) channels: row q*8+i of t9, q = up_idx[pp]
                q = up_idx[pp]
                for i in range(8):
                    m4a[q * 8 + i, o * 4 + p] += wd4[o, i, kk]
                # jlrs channels: row pp*6+j of jlt
                for j in range(6):
                    m4b[pp * 6 + j, o * 4 + p] += wd4[o, 8 + j, kk]
    put("d4a", m4a)
    put("d4b", m4b)
    put("d4c", np.repeat(g("b_d4"), 4)[None, :])  # [1, 24] (o*4+p)

    # bias columns (tall)
    put("b_c1", np.tile(g("b_c1"), 4)[:, None])
    put("b_c2", np.tile(g("b_c2"), 4)[:, None])
    put("b_c3", np.tile(g("b_c3"), 2)[:, None])
    put("b_c4", g("b_c4")[:, None])
    put("b_e1", g("b_e1")[:, None])
    put("b_e2", g("b_e2")[:, None])
    put("b_d1", np.tile(g("b_d1"), 3)[:, None])
    put("b_d2", np.tile(g("b_d2"), 3)[:, None])
    put("b_d3", np.tile(g("b_d3"), 3)[:, None])

    # x-derived blocks (pure relayout of x)
    x = np.asarray(inp["x"], np.float32)[0]
    z2 = np.zeros(2, np.float32)
    jl = np.concatenate([z2, x[7:29]]).reshape(6, 4)    # [ch, pos]
    jd = np.concatenate([z2, x[35:57]]).reshape(6, 4)
    jcat = np.concatenate([jl, jd], 0)                  # [12, 4]
    put("jt", jcat.T.reshape(48, 1))                    # tall: p*12+c
    put("jlt", jl.T.reshape(24, 1))                     # tall: p*6+c
    # atan2 lanes: row0 = [n=qz, d=qw], row1 = [n=qx, d=qy]
    put("quat", np.array([[x[6], x[3]], [x[4], x[5]]], np.float32))
    put("obsd", np.array([[x[34]]], np.float32))
    put("ones", np.array([[1.0]], np.float32))
    return W


# ---------------------------------------------------------------------------
# device program
# ---------------------------------------------------------------------------


def build():
    nc = bacc.Bacc("TRN2", target_bir_lowering=False, debug=False)
    wpack_t = nc.dram_tensor("wpack", [_ROWS, _COLS], F32, kind="ExternalInput")
    out_t = nc.dram_tensor("out", [24], F32, kind="ExternalOutput")

    with tile.TileContext(nc) as tc:
        with (
            tc.tile_pool(name="sb", bufs=1) as sb,
            tc.tile_pool(name="pp", bufs=4, space="PSUM") as pp,
            tc.tile_pool(name="pe1", bufs=1, space="PSUM") as pe1,
            tc.tile_pool(name="pd4", bufs=1, space="PSUM") as pd4,
        ):
            w = sb.tile([_ROWS, _COLS], F32)
            t1 = sb.tile([16, 1], F32)
            t2 = sb.tile([32, 1], F32)
            t3 = sb.tile([16, 1], F32)
            t4 = sb.tile([8, 1], F32)
            t5 = sb.tile([8, 1], F32)
            t6 = sb.tile([8, 1], F32)
            t7 = sb.tile([12, 1], F32)
            t8 = sb.tile([12, 1], F32)
            t9 = sb.tile([24, 1], F32)
            pex = sb.tile([2, 1], F32)
            pss = sb.tile([2, 8], F32)
            wrm = sb.tile([1, 2], F32)
            stg = sb.tile([24, 1], F32)

            # single input DMA (SP HWDGE)
            nc.sync.dma_start(w[:, :], wpack_t[:, :])

            # Warm the ACT table set immediately: Arctan pins the set that
            # also holds Tanh and Sign -> exactly one table load, fully
            # overlapped with the input DMA.
            zero_c = nc.const_aps.aps[(F32, 0.0)]
            nc.scalar.activation(
                wrm[:, 1:2], zero_c[0:1, 0:1], AF.Arctan, bias=zero_c[0:1, 0:1]
            )

            def wap(name, h=None, w_=None):
                r, c = _POS[name]
                if h is None:
                    h, w_ = _MM[name] if name in _MM else (None, None)
                return w[r : r + h, c : c + w_]

            def mmap(name):
                h, w_ = _MM[name]
                r, c = _POS[name]
                return w[r : r + h, c : c + w_]

            def bap(name):
                h = _BIAS[name]
                r, c = _POS["b_" + name]
                return w[r : r + h, c : c + 1]

            # psi = atan2(qz,qw) + atan2(qx,qy) on DVE/ACT lanes 0:2.
            # atan2(n,d) = arctan(n/d) + pi*sign(n)*[d<0]
            qr, qc = _POS["quat"]
            n_ap = w[qr : qr + 2, qc : qc + 1]
            d_ap = w[qr : qr + 2, qc + 1 : qc + 2]
            nc.vector.reciprocal(pss[0:2, 0:1], d_ap)
            nc.vector.tensor_tensor(pss[0:2, 1:2], n_ap, pss[0:2, 0:1], ALU.mult)
            nc.scalar.activation(
                pss[0:2, 2:3], pss[0:2, 1:2], AF.Arctan, bias=zero_c[0:2, 0:1]
            )
            nc.scalar.activation(pss[0:2, 3:4], n_ap, AF.Sign, bias=zero_c[0:2, 0:1])
            nc.vector.tensor_scalar(pss[0:2, 4:5], d_ap, 0.0, None, ALU.is_lt)
            nc.vector.tensor_scalar(
                pss[0:2, 5:6], pss[0:2, 4:5], pss[0:2, 3:4], float(np.pi),
                ALU.mult, ALU.mult,
            )
            nc.vector.tensor_tensor(pex[0:2, 0:1], pss[0:2, 2:3], pss[0:2, 5:6],
                                    ALU.add)

            def mm(ps, name, rhs, start, stop):
                h, w_ = _MM[name]
                nc.tensor.matmul(ps[0:w_, 0:1], mmap(name), rhs, start=start,
                                 stop=stop)

            def layer(name, ps, rhs, out):
                mm(ps, name, rhs, True, True)
                nc.scalar.activation(out, ps[0 : out.shape[0], 0:1], AF.Tanh,
                                     bias=bap(name))

            # d4's jlrs/bias matmuls depend only on the input DMA -- run
            # them first so only the d4a matmul sits on the critical tail.
            ps4 = pd4.tile([24, 1], F32, tag="d4")
            mm(ps4, "d4b", wap("jlt", 24, 1), True, False)
            mm(ps4, "d4c", wap("ones", 1, 1), False, False)

            ps = pp.tile([16, 1], F32, tag="ps")
            layer("c1", ps, wap("jt", 48, 1), t1[:, :])
            ps = pp.tile([32, 1], F32, tag="ps")
            layer("c2", ps, t1[:, :], t2[:, :])
            ps = pp.tile([16, 1], F32, tag="ps")
            layer("c3", ps, t2[:, :], t3[:, :])
            ps = pp.tile([8, 1], F32, tag="ps")
            layer("c4", ps, t3[:, :], t4[:, :])

            # e1: three accumulating parts (conv on emb, psi lanes, obsd)
            ps5 = pe1.tile([8, 1], F32, tag="e1")
            mm(ps5, "e1b", pex[:, :], True, False)
            mm(ps5, "e1c", wap("obsd", 1, 1), False, False)
            mm(ps5, "e1a", t4[:, :], False, True)
            nc.scalar.activation(t5[:, :], ps5[0:8, 0:1], AF.Tanh, bias=bap("e1"))

            ps = pp.tile([8, 1], F32, tag="ps")
            layer("e2", ps, t5[:, :], t6[:, :])
            ps = pp.tile([12, 1], F32, tag="ps")
            layer("d1", ps, t6[:, :], t7[:, :])
            ps = pp.tile([12, 1], F32, tag="ps")
            layer("d2", ps, t7[:, :], t8[:, :])
            ps = pp.tile([24, 1], F32, tag="ps")
            layer("d3", ps, t8[:, :], t9[:, :])

            # final linear layer: accumulate the d4a part, then PSUM -> SBUF
            # (free-size-1 copy) -> DRAM
            mm(ps4, "d4a", t9[:, :], False, True)
            nc.vector.tensor_copy(stg[0:24, 0:1], ps4[0:24, 0:1])
            nc.sync.dma_start(out_t[:], stg[0:24, 0:1])

    nc.compile()
    return nc


_NC = None


def _get_nc():
    global _NC
    if _NC is None:
        _NC = build()
    return _NC


def make_in_map(inputs):
    return {"wpack": pack_all(inputs)}


def kernel(**inputs) -> np.ndarray:
    nc = _get_nc()
    res = run_bass_kernel_spmd(nc, [make_in_map(inputs)], core_ids=[0])
    acts = np.asarray(res.results[0]["out"], np.float32).reshape(24)
    return np.ascontiguousarray(acts[None, 2:])
